# revision 2
# baseline (speedup 1.0000x reference)
"""GAT attention head (nn_AttnHead) on 8 Trainium2 NeuronCores, v3.

Exponent restructure to make ACT (the exp engine) the only bottleneck:
  softmax rows are invariant to subtracting f1_i, so the required
  exponent is  max(f2_j, -0.99*f1_i + 0.01*f2_j)
            =  0.99 * max( (-f1_i) + K1*f2_j,  K2*f2_j ),
  K1 = 1/0.99 - 1, K2 = 1/0.99.  K1*f2 / K2*f2 fall out of the
  projection as two extra host-scaled W1 columns (b3 enters via a
  rank-1 bias-row matmul), so the whole attention inner loop is ONE
  DVE tensor_scalar (add,max with two per-partition vector scalars)
  per 128x512 chunk plus ONE wide bias-free exp per 16 chunks --
  amortizing the ~350cyc/instr ACT overhead that dominated v2 (64
  narrow exps + per-chunk biases + 2-op DVE lrelu chains).
  bias_mat is all-zeros per the harness spec; the fast path never
  reads it (halving DMA).  kernel() checks host-side and falls back
  to the v2 program (kept verbatim below) if bias_mat is nonzero.
"""

import numpy as np

import concourse.bass as bass
import concourse.bacc as bacc
import concourse.tile as tile
from concourse import mybir
from concourse.bass_utils import run_bass_kernel_spmd

B, N, F, O = 2, 4096, 256, 64
P = 128
R = 8                 # cores
NL = N // R           # 512 local query rows per core
NB = NL // P          # 4 own row blocks
JC = N // P           # 32 key chunks of 128
EXT = O + 2           # projection channels: [sf(64) | K1*f2 | K2*f2]
SLOPE = 0.01
K2C = 1.0 / (1.0 - SLOPE)
K1C = K2C - 1.0
EPS = 1e-5
f32 = mybir.dt.float32
f32r = mybir.dt.float32r
bf16 = mybir.dt.bfloat16
AFT = mybir.ActivationFunctionType
ALU = mybir.AluOpType

_CACHE = {}


def _patch_one_table(nc):
    # All activation funcs this kernel uses live together in the
    # 'natural_log_exp_and_others' set; the default chooser maps each
    # func to the FIRST set containing it (exp->0, ln->5), forcing ~4
    # table reloads per rep. Remove our funcs from every other set so
    # the chooser lands on the covering set and the load hoists out of
    # the loop entirely.
    import types
    import bass_rust as _bass_rust
    from concourse.hw_specs import get_activation_tables
    from concourse import mybir as _mb

    _mine = {AFT.Exp, AFT.Ln, AFT.Relu, AFT.Square, AFT.Identity, AFT.Copy}
    _keep = "natural_log_exp_and_others"

    def _patched_insert_act_table_loads(self):
        has_activation = any(
            isinstance(i, _mb.InstActivation)
            for b in self.main_func.blocks
            for i in b.instructions
        )
        if not has_activation:
            return
        tables = [
            (nm, (s if nm == _keep else (s - _mine)))
            for nm, s in get_activation_tables(self.m.arch).items()
        ]
        _bass_rust.insert_act_table_loads(self, tables)

    nc.insert_act_table_loads = types.MethodType(
        _patched_insert_act_table_loads, nc
    )


def _build_program(n_reps=1, gsz=16, ilv=True, dma_only=False, no_cc=False,
                   flush_frac=0.75, spread_frac=0.75, lp=2):
    """lp: 0 = u f32 / e f32r / sfa f32r (max precision)
           1 = u f32 / e bf16 / sfa bf16
           2 = u bf16 / e bf16 / sfa bf16 (fastest)"""
    key = ("v3", n_reps, gsz, ilv, dma_only, no_cc, flush_frac, spread_frac, lp)
    if key in _CACHE:
        return _CACHE[key]
    wdt = bf16 if lp >= 2 else f32
    edt = bf16 if lp >= 1 else f32r
    adt = bf16 if lp >= 1 else f32r

    nc = bacc.Bacc("TRN2", target_bir_lowering=False, debug=False, num_devices=R)
    _patch_one_table(nc)

    seqt_in = nc.dram_tensor("seqt", [B, 2, P, N], bf16, kind="ExternalInput").ap()
    sto_in = nc.dram_tensor("seqt_own", [B, 2, P, NL], bf16, kind="ExternalInput").ap()
    w1e_in = nc.dram_tensor("w1ext", [2, P, EXT], bf16, kind="ExternalInput").ap()
    w12_in = nc.dram_tensor("w1w2", [2, P, 1], bf16, kind="ExternalInput").ap()
    gam_in = nc.dram_tensor("gamma_c", [O, 1], f32, kind="ExternalInput").ap()
    bet_in = nc.dram_tensor("beta_c", [O, 1], f32, kind="ExternalInput").ap()
    sc_in = nc.dram_tensor("scalars", [1, 3], f32, kind="ExternalInput").ap()
    out_ext = nc.dram_tensor("out_loc", [B, NL, O], f32, kind="ExternalOutput").ap()

    st_in = nc.dram_tensor("st_in", [O, 2], f32)
    st_out = nc.dram_tensor("st_out", [R * O, 2], f32, addr_space="Shared")

    ident_d = nc.inline_tensor(np.eye(P, dtype=np.float32), name="ident")
    rg = [list(range(R))]
    NG = JC // gsz            # exp groups per batch
    ngroups = B * NG

    with tile.TileContext(nc, num_cores=R) as tc:
        with (
            tc.tile_pool(name="consts", bufs=1) as consts,
            tc.tile_pool(name="sfx", bufs=2) as sfx,
            tc.tile_pool(name="perb", bufs=3) as perb,
            tc.tile_pool(name="persist", bufs=2) as persist,
            tc.tile_pool(name="tbig", bufs=4) as tbig,
            tc.tile_pool(name="seqso", bufs=3) as seqso,
            tc.tile_pool(name="seqstq", bufs=9) as seqstq,
            tc.tile_pool(name="work", bufs=2) as work,
            tc.tile_pool(name="tailp", bufs=2) as tailp,
            tc.tile_pool(name="outp", bufs=2) as outp,
            tc.tile_pool(name="ps_proj", bufs=2, space="PSUM") as ps_proj,
            tc.tile_pool(name="ps_agg", bufs=2, space="PSUM") as ps_agg,
            tc.tile_pool(name="ps_misc", bufs=2, space="PSUM") as ps_misc,
        ):
            # ---------- constants ----------
            ident = consts.tile([P, P], f32)
            nc.sync.dma_start(out=ident, in_=ident_d.ap())
            w1e0 = consts.tile([P, EXT], bf16)
            nc.sync.dma_start(out=w1e0, in_=w1e_in[0])
            w1e1 = consts.tile([P, EXT], bf16)
            nc.sync.dma_start(out=w1e1, in_=w1e_in[1])
            w12_0 = consts.tile([P, 1], bf16)
            nc.sync.dma_start(out=w12_0, in_=w12_in[0])
            w12_1 = consts.tile([P, 1], bf16)
            nc.sync.dma_start(out=w12_1, in_=w12_in[1])
            gam = consts.tile([O, 1], f32)
            nc.sync.dma_start(out=gam, in_=gam_in)
            bet = consts.tile([O, 1], f32)
            nc.sync.dma_start(out=bet, in_=bet_in)
            sc_t = consts.tile([1, 3], f32)
            nc.sync.dma_start(out=sc_t, in_=sc_in)
            b2t = sc_t[0:1, 0:1]
            ones_r = consts.tile([1, P], f32)
            nc.scalar.activation(ones_r, ident[0:1, :], AFT.Copy, bias=1.0, scale=0.0)
            ones_o = consts.tile([1, O], f32)
            nc.scalar.activation(ones_o, ident[0:1, 0:O], AFT.Copy, bias=1.0, scale=0.0)
            eps_t = consts.tile([O, 1], f32)
            nc.scalar.activation(eps_t, ident[0:O, 0:1], AFT.Copy, bias=EPS, scale=0.0)
            # bias row for the projection: [1, 4, EXT] with b3*K1 at col
            # O and b3*K2 at col O+1 of each cc block, zero elsewhere.
            brow = consts.tile([1, 4, EXT], f32)
            nc.vector.memset(brow, 0.0)
            nc.vector.tensor_scalar(
                brow[:, :, O:O + 1], ident[0:1, 0:4], 0.0, sc_t[0:1, 1:2],
                ALU.mult, ALU.add,
            )
            nc.vector.tensor_scalar(
                brow[:, :, O + 1:O + 2], ident[0:1, 0:4], 0.0, sc_t[0:1, 2:3],
                ALU.mult, ALU.add,
            )

            def make_phaseA():
                """Phase A as a list of closures; each emits a small chunk."""
                state = {"f1_reps": [None, None]}
                chunks = []

                def c_dma():
                    stos, stqs = [], []
                    for b in range(B):
                        sto = seqso.tile([P, 2, NL], bf16, tag="sto")
                        nc.scalar.dma_start(out=sto[:, 0, :], in_=sto_in[b, 0])
                        nc.scalar.dma_start(out=sto[:, 1, :], in_=sto_in[b, 1])
                        stos.append(sto)
                    for b in range(B):
                        for g in range(4):
                            stq = seqstq.tile([P, 2, 1024], bf16, tag="stq")
                            nc.sync.dma_start(
                                out=stq[:, 0, :],
                                in_=seqt_in[b, 0, :, g * 1024:(g + 1) * 1024],
                            )
                            nc.sync.dma_start(
                                out=stq[:, 1, :],
                                in_=seqt_in[b, 1, :, g * 1024:(g + 1) * 1024],
                            )
                            stqs.append(stq)
                    state["stos"] = stos
                    state["stqs"] = stqs
                    sfa_t = sfx.tile([P, B, JC, O + 1], adt, tag="sfa")
                    sfb_t = sfx.tile([P, B, JC, 2], f32, tag="sfb")
                    nc.vector.memset(sfa_t[:, :, :, O:O + 1], 1.0)
                    state["sfa"] = sfa_t
                    state["sfb"] = sfb_t
                chunks.append(c_dma)

                def c_f1(b):
                    sto = state["stos"][b]
                    ps_f1 = ps_misc.tile([1, NL], f32, tag="pmisc")
                    nc.tensor.matmul(
                        ps_f1, lhsT=w12_0, rhs=sto[:, 0, :], start=True, stop=False
                    )
                    nc.tensor.matmul(
                        ps_f1, lhsT=w12_1, rhs=sto[:, 1, :], start=False, stop=True
                    )
                    # nf1 = -(f1 + b2), replicated to all partitions
                    f1row = tailp.tile([1, NL], f32, tag="f1row")
                    nc.vector.tensor_scalar(
                        f1row, ps_f1, b2t, -1.0, ALU.add, ALU.mult
                    )
                    ps_rep = ps_misc.tile([P, NL], f32, tag="pmisc")
                    nc.tensor.matmul(
                        ps_rep, lhsT=ones_r, rhs=f1row, start=True, stop=True
                    )
                    f1_rep = perb.tile([P, NL], wdt, tag="f1_rep")
                    nc.vector.tensor_copy(f1_rep, ps_rep)
                    state["f1_reps"][b] = f1_rep
                chunks.append(lambda: c_f1(0))
                chunks.append(lambda: c_f1(1))

                def c_proj(b, g, cg):
                    stq = state["stqs"][b * 4 + g]
                    sfa, sfb = state["sfa"], state["sfb"]
                    ps_p = ps_proj.tile([P, 4, EXT], f32, tag="ps_p")
                    nc.tensor.matmul(
                        ps_p, lhsT=ones_r, rhs=brow, start=True, stop=False
                    )
                    for cc in range(4):
                        col = cg * 4 + cc
                        nc.tensor.matmul(
                            ps_p[:, cc, :],
                            lhsT=stq[:, 0, col * P:(col + 1) * P],
                            rhs=w1e0,
                            start=False, stop=False,
                        )
                        nc.tensor.matmul(
                            ps_p[:, cc, :],
                            lhsT=stq[:, 1, col * P:(col + 1) * P],
                            rhs=w1e1,
                            start=False, stop=True,
                        )
                    jc0 = g * 8 + cg * 4
                    nc.vector.tensor_copy(
                        sfa[:, b, jc0:jc0 + 4, 0:O], ps_p[:, :, 0:O]
                    )
                    nc.vector.tensor_copy(
                        sfb[:, b, jc0:jc0 + 4, :], ps_p[:, :, O:O + 2]
                    )
                for b in range(B):
                    for g in range(4):
                        for cg in range(2):
                            chunks.append(
                                lambda b=b, g=g, cg=cg: c_proj(b, g, cg)
                            )
                return chunks, state

            pending = [None]

            def _main(state, next_chunks):
                sfa, sfb = state["sfa"], state["sfb"]
                f1_reps = state["f1_reps"]
                ps_ags = []
                for _b in range(B):
                    ps_ag = ps_agg.tile([O + 1, NL], f32, tag="agg")
                    ps_ags.append(ps_ag)
                if ilv:
                    sched = [(b, h) for h in range(NG) for b in range(B)]
                else:
                    sched = [(b, h) for b in range(B) for h in range(NG)]
                nq = list(next_chunks)
                ncn = len(nq)
                emitted = 0
                nslots = ngroups * gsz
                spread = max(1, int(nslots * spread_frac))
                flush_at = max(1, int(ngroups * flush_frac))
                for gi, (b, h) in enumerate(sched):
                    if gi == flush_at and pending[0] is not None:
                        pending[0]()
                        pending[0] = None
                    u = work.tile([P, gsz, NL], wdt, tag="u")
                    for j in range(gsz):
                        tsi = gi * gsz + j
                        if ncn:
                            want = min(ncn, (tsi * ncn) // spread + 1)
                            while emitted < want:
                                nq[emitted]()
                                emitted += 1
                        jc = h * gsz + j
                        nc.vector.tensor_scalar(
                            u[:, j, :], f1_reps[b],
                            sfb[:, b, jc, 0:1], sfb[:, b, jc, 1:2],
                            ALU.add, ALU.max,
                        )
                    e = work.tile([P, gsz, NL], edt, tag="e")
                    nc.scalar.activation(e, u, AFT.Exp, scale=(1.0 - SLOPE))
                    for j in range(gsz):
                        jc = h * gsz + j
                        nc.tensor.matmul(
                            ps_ags[b],
                            lhsT=sfa[:, b, jc, :],
                            rhs=e[:, j, :],
                            start=(jc == 0), stop=(jc == JC - 1),
                        )
                while emitted < ncn:
                    nq[emitted]()
                    emitted += 1

                # ---------- softmax divide + BN stats + AllGather ----------
                valsT = persist.tile([O, B * NL], f32, tag="valsT")
                ssums = tailp.tile([O, 2], f32, tag="ssums")
                ssqs = tailp.tile([O, 2], f32, tag="ssqs")
                lnds = []
                for b in range(B):
                    lnd = tailp.tile([1, NL], f32, tag="lnd")
                    nc.scalar.activation(lnd, ps_ags[b][O:O + 1, :], AFT.Ln)
                    lnds.append(lnd)
                for b in range(B):
                    rrow = tailp.tile([1, NL], f32, tag="rrow")
                    nc.scalar.activation(rrow, lnds[b], AFT.Exp, scale=-1.0)
                    ps_bc = ps_misc.tile([O, NL], f32, tag="pmisc")
                    nc.tensor.matmul(ps_bc, lhsT=ones_o, rhs=rrow, start=True, stop=True)
                    nums = tailp.tile([O, NL], f32, tag="nums")
                    nc.vector.tensor_copy(nums, ps_ags[b][0:O, :])
                    nc.vector.scalar_tensor_tensor(
                        valsT[:, b * NL:(b + 1) * NL], nums, 1.0, ps_bc,
                        ALU.mult, ALU.mult, accum_out=ssums[:, b:b + 1],
                    )

                sqt = tbig.tile([O, B * NL], f32, tag="tbig")
                for b in range(B):
                    nc.vector.scalar_tensor_tensor(
                        sqt[:, b * NL:(b + 1) * NL],
                        valsT[:, b * NL:(b + 1) * NL], 1.0,
                        valsT[:, b * NL:(b + 1) * NL],
                        ALU.mult, ALU.mult, accum_out=ssqs[:, b:b + 1],
                    )
                stt = tailp.tile([O, 2], f32, tag="stt")
                nc.vector.tensor_reduce(
                    stt[:, 0:1], ssums, axis=mybir.AxisListType.X, op=ALU.add
                )
                nc.vector.tensor_reduce(
                    stt[:, 1:2], ssqs, axis=mybir.AxisListType.X, op=ALU.add
                )
                nc.scalar.dma_start(out=st_in.ap(), in_=stt)
                if not no_cc:
                    nc.gpsimd.collective_compute(
                        "AllGather", ALU.bypass, replica_groups=rg,
                        ins=[st_in.ap()], outs=[st_out.ap()],
                    )

                def _tail(valsT=valsT):
                    tot3 = tailp.tile([O, 2, R], f32, tag="tot3")
                    nc.scalar.dma_start(
                        out=tot3,
                        in_=bass.AP(
                            tensor=st_out.ap().tensor, offset=0,
                            ap=[[2, O], [1, 2], [2 * O, R]],
                        ),
                    )
                    tot = tailp.tile([O, 2], f32, tag="tot")
                    nc.vector.tensor_reduce(
                        tot, tot3, axis=mybir.AxisListType.X, op=ALU.add
                    )
                    mean = tailp.tile([O, 1], f32, tag="mean")
                    nc.vector.tensor_scalar_mul(mean, tot[:, 0:1], 1.0 / (B * N))
                    ex2 = tailp.tile([O, 1], f32, tag="ex2")
                    nc.vector.tensor_scalar_mul(ex2, tot[:, 1:2], 1.0 / (B * N))
                    msq = tailp.tile([O, 1], f32, tag="msq")
                    nc.scalar.activation(msq, mean, AFT.Square)
                    var = tailp.tile([O, 1], f32, tag="var")
                    nc.vector.tensor_tensor(var, ex2, msq, ALU.subtract)
                    lnv = tailp.tile([O, 1], f32, tag="lnv")
                    nc.scalar.activation(lnv, var, AFT.Ln, bias=eps_t)
                    istd = tailp.tile([O, 1], f32, tag="istd")
                    nc.scalar.activation(istd, lnv, AFT.Exp, scale=-0.5)
                    scal = tailp.tile([O, 1], f32, tag="scal")
                    nc.vector.tensor_tensor(scal, istd, gam, ALU.mult)
                    mscal = tailp.tile([O, 1], f32, tag="mscal")
                    nc.vector.tensor_tensor(mscal, mean, scal, ALU.mult)
                    shift = tailp.tile([O, 1], f32, tag="shift")
                    nc.vector.tensor_tensor(shift, bet, mscal, ALU.subtract)

                    ret = tbig.tile([O, B * NL], f32, tag="tbig")
                    nc.vector.tensor_scalar(
                        ret, valsT, scal, shift, ALU.mult, ALU.add
                    )
                    pos = tbig.tile([O, B * NL], f32, tag="tbig")
                    nc.vector.tensor_scalar_max(pos, ret, 0.0)
                    mng = tbig.tile([O, B * NL], f32, tag="tbig")
                    nc.vector.tensor_scalar_min(mng, ret, 0.0)
                    em = tbig.tile([O, B * NL], f32, tag="tbig")
                    nc.scalar.activation(em, mng, AFT.Exp)
                    fin = tbig.tile([O, B * NL], f32, tag="tbig")
                    nc.vector.scalar_tensor_tensor(fin, pos, -1.0, em, ALU.add, ALU.add)

                    oT = outp.tile([P, B, NB, O], f32, tag="oT")
                    for b in range(B):
                        for nb in range(NB):
                            c0 = b * NL + nb * P
                            ps_oT = ps_misc.tile([P, O], f32, tag="pmisc")
                            nc.tensor.transpose(
                                ps_oT, fin[:, c0:c0 + P], ident[0:O, 0:O]
                            )
                            nc.vector.tensor_copy(oT[:, b, nb, :], ps_oT)
                    nc.scalar.dma_start(
                        out=bass.AP(
                            tensor=out_ext.tensor, offset=0,
                            ap=[[O, P], [NL * O, B], [P * O, NB], [1, O]],
                        ),
                        in_=oT,
                    )

                pending[0] = _tail

            def _dma_body():
                for b in range(B):
                    sto = seqso.tile([P, 2, NL], bf16, tag="sto")
                    nc.scalar.dma_start(out=sto[:, 0, :], in_=sto_in[b, 0])
                    nc.scalar.dma_start(out=sto[:, 1, :], in_=sto_in[b, 1])
                    for g in range(4):
                        stq = seqstq.tile([P, 2, 1024], bf16, tag="stq")
                        nc.sync.dma_start(
                            out=stq[:, 0, :],
                            in_=seqt_in[b, 0, :, g * 1024:(g + 1) * 1024],
                        )
                        nc.sync.dma_start(
                            out=stq[:, 1, :],
                            in_=seqt_in[b, 1, :, g * 1024:(g + 1) * 1024],
                        )
                oT = outp.tile([P, B, NB, O], f32, tag="oT")
                nc.vector.memset(oT, 0.0)
                nc.gpsimd.dma_start(
                    out=bass.AP(
                        tensor=out_ext.tensor, offset=0,
                        ap=[[O, P], [NL * O, B], [P * O, NB], [1, O]],
                    ),
                    in_=oT,
                )

            curA = None
            for _rep in range(n_reps):
                if dma_only:
                    _dma_body()
                    continue
                if curA is None:
                    chunks, curA = make_phaseA()
                    for c in chunks:
                        c()
                if _rep + 1 < n_reps:
                    nchunks, nstate = make_phaseA()
                else:
                    nchunks, nstate = [], None
                _main(curA, nchunks)
                curA = nstate
            if pending[0] is not None:
                pending[0]()
                pending[0] = None

    nc.compile()
    _CACHE[key] = nc
    return nc


def _prep_inputs(seq, bias_mat, W1, w2, b2, w3, b3, gamma, beta):
    import ml_dtypes
    nbf = ml_dtypes.bfloat16

    seq = np.asarray(seq, dtype=np.float32)
    W1 = np.asarray(W1, dtype=np.float32)
    w2 = np.asarray(w2, dtype=np.float32)
    w3 = np.asarray(w3, dtype=np.float32)

    # seq^T [B, F, N] -> [B, 2, 128, N], bf16
    seqT = np.ascontiguousarray(
        seq.transpose(0, 2, 1).reshape(B, 2, P, N).astype(nbf)
    )
    # W1 extended: [sf(64) | K1*W1^T w3 | K2*W1^T w3] -> [2, 128, EXT]
    w1t = W1.T  # [F, O]
    f2col = (w1t @ w3).reshape(F, 1)
    w1ext = np.concatenate([w1t, f2col * K1C, f2col * K2C], axis=1)
    w1ext = np.ascontiguousarray(w1ext.reshape(2, P, EXT).astype(nbf))
    w1w2 = np.ascontiguousarray((w1t @ w2).reshape(2, P, 1).astype(nbf))

    gam = np.asarray(gamma, dtype=np.float32).reshape(O, 1)
    bet = np.asarray(beta, dtype=np.float32).reshape(O, 1)
    sc = np.array(
        [[float(b2), float(b3) * K1C, float(b3) * K2C]], dtype=np.float32
    )

    in_maps = []
    for c in range(R):
        rows = slice(c * NL, (c + 1) * NL)
        sto = np.ascontiguousarray(seqT[:, :, :, rows])
        in_maps.append({
            "seqt": seqT,
            "seqt_own": sto,
            "w1ext": w1ext,
            "w1w2": w1w2,
            "gamma_c": gam,
            "beta_c": bet,
            "scalars": sc,
        })
    return in_maps


def kernel(seq, bias_mat, W1, w2, b2, w3, b3, gamma, beta):
    if np.asarray(bias_mat).any():
        in_maps = _prep_inputs_v2(seq, bias_mat, W1, w2, b2, w3, b3, gamma, beta)
        nc = _build_program_v2()
    else:
        in_maps = _prep_inputs(seq, bias_mat, W1, w2, b2, w3, b3, gamma, beta)
        nc = _build_program()
    res = run_bass_kernel_spmd(nc, in_maps, core_ids=list(range(R)))
    out = np.concatenate([res.results[c]["out_loc"] for c in range(R)], axis=1)
    return out


# ---------------------------------------------------------------------------
# v2 fallback (streams bias_mat): used only when bias_mat has nonzeros.
# ---------------------------------------------------------------------------

def _build_program_v2(n_reps=1, relu_m=5, ilv=True, flush_at=48, spread=56):
    key = ("v2", n_reps, relu_m, ilv, flush_at, spread)
    wdt = f32
    edt = f32r
    if key in _CACHE:
        return _CACHE[key]

    nc = bacc.Bacc("TRN2", target_bir_lowering=False, debug=False, num_devices=R)
    _patch_one_table(nc)

    seqt_in = nc.dram_tensor("seqt", [B, 2, P, N], bf16, kind="ExternalInput").ap()
    sto_in = nc.dram_tensor("seqt_own", [B, 2, P, NL], bf16, kind="ExternalInput").ap()
    bias_in = nc.dram_tensor("bias_t", [B, P, JC, NL], bf16, kind="ExternalInput").ap()
    w1e_in = nc.dram_tensor("w1ext", [2, P, EXT], bf16, kind="ExternalInput").ap()
    w12_in = nc.dram_tensor("w1w2", [2, P, 1], bf16, kind="ExternalInput").ap()
    gam_in = nc.dram_tensor("gamma_c", [O, 1], f32, kind="ExternalInput").ap()
    bet_in = nc.dram_tensor("beta_c", [O, 1], f32, kind="ExternalInput").ap()
    sc_in = nc.dram_tensor("scalars", [1, 2], f32, kind="ExternalInput").ap()
    out_ext = nc.dram_tensor("out_loc", [B, NL, O], f32, kind="ExternalOutput").ap()

    st_in = nc.dram_tensor("st_in", [O, 2], f32)
    st_out = nc.dram_tensor("st_out", [R * O, 2], f32, addr_space="Shared")

    ident_d = nc.inline_tensor(np.eye(P, dtype=np.float32), name="ident")
    rg = [list(range(R))]

    with tile.TileContext(nc, num_cores=R) as tc:
        with (
            tc.tile_pool(name="consts", bufs=1) as consts,
            tc.tile_pool(name="sfx", bufs=2) as sfx,
            tc.tile_pool(name="perb", bufs=3) as perb,
            tc.tile_pool(name="persist", bufs=2) as persist,
            tc.tile_pool(name="tbig", bufs=4) as tbig,
            tc.tile_pool(name="biasg", bufs=4) as biasg,
            tc.tile_pool(name="seqso", bufs=3) as seqso,
            tc.tile_pool(name="seqstq", bufs=9) as seqstq,
            tc.tile_pool(name="work", bufs=5) as work,
            tc.tile_pool(name="tailp", bufs=2) as tailp,
            tc.tile_pool(name="outp", bufs=2) as outp,
            tc.tile_pool(name="ps_proj", bufs=2, space="PSUM") as ps_proj,
            tc.tile_pool(name="ps_agg", bufs=2, space="PSUM") as ps_agg,
            tc.tile_pool(name="ps_misc", bufs=2, space="PSUM") as ps_misc,
        ):
            ident = consts.tile([P, P], f32)
            nc.sync.dma_start(out=ident, in_=ident_d.ap())
            w1e0 = consts.tile([P, EXT], bf16)
            nc.sync.dma_start(out=w1e0, in_=w1e_in[0])
            w1e1 = consts.tile([P, EXT], bf16)
            nc.sync.dma_start(out=w1e1, in_=w1e_in[1])
            w12_0 = consts.tile([P, 1], bf16)
            nc.sync.dma_start(out=w12_0, in_=w12_in[0])
            w12_1 = consts.tile([P, 1], bf16)
            nc.sync.dma_start(out=w12_1, in_=w12_in[1])
            gam = consts.tile([O, 1], f32)
            nc.sync.dma_start(out=gam, in_=gam_in)
            bet = consts.tile([O, 1], f32)
            nc.sync.dma_start(out=bet, in_=bet_in)
            b2t = consts.tile([1, 1], f32)
            nc.sync.dma_start(out=b2t, in_=sc_in[0:1, 0:1])
            b3r = consts.tile([P, 1], f32)
            nc.gpsimd.dma_start(
                out=b3r,
                in_=bass.AP(tensor=sc_in.tensor, offset=1, ap=[[0, P], [1, 1]]),
            )
            ones_r = consts.tile([1, P], f32)
            nc.scalar.activation(ones_r, ident[0:1, :], AFT.Copy, bias=1.0, scale=0.0)
            ones_o = consts.tile([1, O], f32)
            nc.scalar.activation(ones_o, ident[0:1, 0:O], AFT.Copy, bias=1.0, scale=0.0)
            eps_t = consts.tile([O, 1], f32)
            nc.scalar.activation(eps_t, ident[0:O, 0:1], AFT.Copy, bias=EPS, scale=0.0)
            mb3r = consts.tile([P, 1], f32)
            nc.vector.tensor_scalar_mul(mb3r, b3r, -1.0)

            def make_phaseA():
                state = {"f1_reps": [None, None]}
                chunks = []

                def c_dma():
                    stos, stqs = [], []
                    for b in range(B):
                        sto = seqso.tile([P, 2, NL], bf16, tag="sto")
                        nc.scalar.dma_start(out=sto[:, 0, :], in_=sto_in[b, 0])
                        nc.scalar.dma_start(out=sto[:, 1, :], in_=sto_in[b, 1])
                        stos.append(sto)
                    for b in range(B):
                        for g in range(4):
                            stq = seqstq.tile([P, 2, 1024], bf16, tag="stq")
                            nc.scalar.dma_start(
                                out=stq[:, 0, :],
                                in_=seqt_in[b, 0, :, g * 1024:(g + 1) * 1024],
                            )
                            nc.scalar.dma_start(
                                out=stq[:, 1, :],
                                in_=seqt_in[b, 1, :, g * 1024:(g + 1) * 1024],
                            )
                            stqs.append(stq)
                    state["stos"] = stos
                    state["stqs"] = stqs
                    sfa_t = sfx.tile([P, B, JC, O + 1], f32r, tag="sfa")
                    sfb_t = sfx.tile([P, B, JC, 2], f32, tag="sfb")
                    state["sfa"] = sfa_t
                    state["sfb"] = sfb_t
                chunks.append(c_dma)

                def c_f1(b):
                    sto = state["stos"][b]
                    ps_f1 = ps_misc.tile([1, NL], f32, tag="pmisc")
                    nc.tensor.matmul(
                        ps_f1, lhsT=w12_0, rhs=sto[:, 0, :], start=True, stop=False
                    )
                    nc.tensor.matmul(
                        ps_f1, lhsT=w12_1, rhs=sto[:, 1, :], start=False, stop=True
                    )
                    f1row = tailp.tile([1, NL], f32, tag="f1row")
                    nc.scalar.activation(f1row, ps_f1, AFT.Identity, bias=b2t)
                    ps_rep = ps_misc.tile([P, NL], f32, tag="pmisc")
                    nc.tensor.matmul(
                        ps_rep, lhsT=ones_r, rhs=f1row, start=True, stop=True
                    )
                    f1_rep = perb.tile([P, NL], wdt, tag="f1_rep")
                    nc.vector.tensor_copy(f1_rep, ps_rep)
                    state["f1_reps"][b] = f1_rep
                chunks.append(lambda: c_f1(0))
                chunks.append(lambda: c_f1(1))

                def c_proj(b, g, cg):
                    stq = state["stqs"][b * 4 + g]
                    sfa, sfb = state["sfa"], state["sfb"]
                    ps_p = ps_proj.tile([P, 4, EXT], f32, tag="ps_p")
                    for cc in range(4):
                        col = cg * 4 + cc
                        nc.tensor.matmul(
                            ps_p[:, cc, :],
                            lhsT=stq[:, 0, col * P:(col + 1) * P],
                            rhs=w1e0,
                            start=True, stop=False,
                        )
                        nc.tensor.matmul(
                            ps_p[:, cc, :],
                            lhsT=stq[:, 1, col * P:(col + 1) * P],
                            rhs=w1e1,
                            start=False, stop=True,
                        )
                    jc0 = g * 8 + cg * 4
                    nc.vector.tensor_copy(
                        sfa[:, b, jc0:jc0 + 4, 0:O], ps_p[:, :, 0:O]
                    )
                    nc.vector.tensor_scalar(
                        sfb[:, b, jc0:jc0 + 4, 0:1],
                        ps_p[:, :, O:O + 1], b3r, 0.0, ALU.add, ALU.add,
                    )
                    nc.vector.tensor_scalar(
                        sfb[:, b, jc0:jc0 + 4, 1:2],
                        ps_p[:, :, O + 1:O + 2], mb3r, 0.0, ALU.add, ALU.add,
                    )
                    nc.vector.tensor_scalar(
                        sfa[:, b, jc0:jc0 + 4, O:O + 1],
                        ps_p[:, :, O:O + 1], 0.0, 1.0, ALU.mult, ALU.add,
                    )
                for b in range(B):
                    for g in range(4):
                        for cg in range(2):
                            chunks.append(
                                lambda b=b, g=g, cg=cg: c_proj(b, g, cg)
                            )
                return chunks, state

            pending = [None]

            def _main(state, next_chunks):
                sfa, sfb = state["sfa"], state["sfb"]
                f1_reps = state["f1_reps"]
                ps_ags = []
                for _b in range(B):
                    ps_ag = ps_agg.tile([O + 1, NL], f32, tag="agg")
                    ps_ags.append(ps_ag)
                bqs = [None, None]
                if ilv:
                    sched = [(b, q, j) for q in range(4) for j in range(8)
                             for b in range(B)]
                else:
                    sched = [(b, q, j) for b in range(B) for q in range(4)
                             for j in range(8)]
                nq = list(next_chunks)
                ncn = len(nq)
                emitted = 0
                for idx, (b, q, j) in enumerate(sched):
                    if ncn:
                        want = min(ncn, (idx * ncn) // spread + 1)
                        while emitted < want:
                            nq[emitted]()
                            emitted += 1
                    if idx == flush_at and pending[0] is not None:
                        pending[0]()
                        pending[0] = None
                    if j == 0:
                        bq = biasg.tile([P, 8, NL], bf16, tag="biasg")
                        nc.sync.dma_start(
                            out=bq, in_=bias_in[b, :, q * 8:(q + 1) * 8, :]
                        )
                        bqs[b] = bq
                    jc = q * 8 + j
                    f2c = sfb[:, b, jc, 0:1]
                    nf2c = sfb[:, b, jc, 1:2]
                    w = work.tile([P, NL], wdt, tag="w")
                    m = idx % 16
                    if m < relu_m:
                        r = work.tile([P, NL], wdt, tag="r")
                        nc.scalar.activation(
                            r, f1_reps[b], AFT.Relu, bias=nf2c, scale=-1.0
                        )
                        nc.vector.scalar_tensor_tensor(
                            w, r, 1.0 - SLOPE, bqs[b][:, j, :], ALU.mult, ALU.add
                        )
                    else:
                        t = work.tile([P, NL], wdt, tag="t")
                        nc.vector.tensor_scalar(
                            t, f1_reps[b], f2c, -(1.0 - SLOPE), ALU.add, ALU.mult
                        )
                        nc.vector.scalar_tensor_tensor(
                            w, t, 0.0, bqs[b][:, j, :], ALU.max, ALU.add
                        )
                    e = work.tile([P, NL], edt, tag="e")
                    nc.scalar.activation(e, w, AFT.Exp, bias=f2c)
                    nc.tensor.matmul(
                        ps_ags[b],
                        lhsT=sfa[:, b, jc, :],
                        rhs=e,
                        start=(jc == 0), stop=(jc == JC - 1),
                    )
                while emitted < ncn:
                    nq[emitted]()
                    emitted += 1

                valsT = persist.tile([O, B * NL], f32, tag="valsT")
                ssums = tailp.tile([O, 2], f32, tag="ssums")
                ssqs = tailp.tile([O, 2], f32, tag="ssqs")
                lnds = []
                for b in range(B):
                    lnd = tailp.tile([1, NL], f32, tag="lnd")
                    nc.scalar.activation(lnd, ps_ags[b][O:O + 1, :], AFT.Ln)
                    lnds.append(lnd)
                for b in range(B):
                    rrow = tailp.tile([1, NL], f32, tag="rrow")
                    nc.scalar.activation(rrow, lnds[b], AFT.Exp, scale=-1.0)
                    ps_bc = ps_misc.tile([O, NL], f32, tag="pmisc")
                    nc.tensor.matmul(ps_bc, lhsT=ones_o, rhs=rrow, start=True, stop=True)
                    nums = tailp.tile([O, NL], f32, tag="nums")
                    nc.vector.tensor_copy(nums, ps_ags[b][0:O, :])
                    nc.vector.scalar_tensor_tensor(
                        valsT[:, b * NL:(b + 1) * NL], nums, 1.0, ps_bc,
                        ALU.mult, ALU.mult, accum_out=ssums[:, b:b + 1],
                    )

                sqt = tbig.tile([O, B * NL], f32, tag="tbig")
                for b in range(B):
                    nc.vector.scalar_tensor_tensor(
                        sqt[:, b * NL:(b + 1) * NL],
                        valsT[:, b * NL:(b + 1) * NL], 1.0,
                        valsT[:, b * NL:(b + 1) * NL],
                        ALU.mult, ALU.mult, accum_out=ssqs[:, b:b + 1],
                    )
                stt = tailp.tile([O, 2], f32, tag="stt")
                nc.vector.tensor_reduce(
                    stt[:, 0:1], ssums, axis=mybir.AxisListType.X, op=ALU.add
                )
                nc.vector.tensor_reduce(
                    stt[:, 1:2], ssqs, axis=mybir.AxisListType.X, op=ALU.add
                )
                nc.scalar.dma_start(out=st_in.ap(), in_=stt)
                nc.gpsimd.collective_compute(
                    "AllGather", ALU.bypass, replica_groups=rg,
                    ins=[st_in.ap()], outs=[st_out.ap()],
                )

                def _tail(valsT=valsT):
                    tot3 = tailp.tile([O, 2, R], f32, tag="tot3")
                    nc.scalar.dma_start(
                        out=tot3,
                        in_=bass.AP(
                            tensor=st_out.ap().tensor, offset=0,
                            ap=[[2, O], [1, 2], [2 * O, R]],
                        ),
                    )
                    tot = tailp.tile([O, 2], f32, tag="tot")
                    nc.vector.tensor_reduce(
                        tot, tot3, axis=mybir.AxisListType.X, op=ALU.add
                    )
                    mean = tailp.tile([O, 1], f32, tag="mean")
                    nc.vector.tensor_scalar_mul(mean, tot[:, 0:1], 1.0 / (B * N))
                    ex2 = tailp.tile([O, 1], f32, tag="ex2")
                    nc.vector.tensor_scalar_mul(ex2, tot[:, 1:2], 1.0 / (B * N))
                    msq = tailp.tile([O, 1], f32, tag="msq")
                    nc.scalar.activation(msq, mean, AFT.Square)
                    var = tailp.tile([O, 1], f32, tag="var")
                    nc.vector.tensor_tensor(var, ex2, msq, ALU.subtract)
                    lnv = tailp.tile([O, 1], f32, tag="lnv")
                    nc.scalar.activation(lnv, var, AFT.Ln, bias=eps_t)
                    istd = tailp.tile([O, 1], f32, tag="istd")
                    nc.scalar.activation(istd, lnv, AFT.Exp, scale=-0.5)
                    scal = tailp.tile([O, 1], f32, tag="scal")
                    nc.vector.tensor_tensor(scal, istd, gam, ALU.mult)
                    mscal = tailp.tile([O, 1], f32, tag="mscal")
                    nc.vector.tensor_tensor(mscal, mean, scal, ALU.mult)
                    shift = tailp.tile([O, 1], f32, tag="shift")
                    nc.vector.tensor_tensor(shift, bet, mscal, ALU.subtract)

                    ret = tbig.tile([O, B * NL], f32, tag="tbig")
                    nc.scalar.activation(ret, valsT, AFT.Identity, bias=shift, scale=scal)
                    pos = tbig.tile([O, B * NL], f32, tag="tbig")
                    nc.vector.tensor_scalar_max(pos, ret, 0.0)
                    mng = tbig.tile([O, B * NL], f32, tag="tbig")
                    nc.vector.tensor_scalar_min(mng, ret, 0.0)
                    em = tbig.tile([O, B * NL], f32, tag="tbig")
                    nc.scalar.activation(em, mng, AFT.Exp)
                    fin = tbig.tile([O, B * NL], f32, tag="tbig")
                    nc.vector.scalar_tensor_tensor(fin, pos, -1.0, em, ALU.add, ALU.add)

                    oT = outp.tile([P, B, NB, O], f32, tag="oT")
                    for b in range(B):
                        for nb in range(NB):
                            c0 = b * NL + nb * P
                            ps_oT = ps_misc.tile([P, O], f32, tag="pmisc")
                            nc.tensor.transpose(
                                ps_oT, fin[:, c0:c0 + P], ident[0:O, 0:O]
                            )
                            nc.vector.tensor_copy(oT[:, b, nb, :], ps_oT)
                    nc.scalar.dma_start(
                        out=bass.AP(
                            tensor=out_ext.tensor, offset=0,
                            ap=[[O, P], [NL * O, B], [P * O, NB], [1, O]],
                        ),
                        in_=oT,
                    )

                pending[0] = _tail

            curA = None
            for _rep in range(n_reps):
                if curA is None:
                    chunks, curA = make_phaseA()
                    for c in chunks:
                        c()
                if _rep + 1 < n_reps:
                    nchunks, nstate = make_phaseA()
                else:
                    nchunks, nstate = [], None
                _main(curA, nchunks)
                curA = nstate
            if pending[0] is not None:
                pending[0]()
                pending[0] = None

    nc.compile()
    _CACHE[key] = nc
    return nc


def _prep_inputs_v2(seq, bias_mat, W1, w2, b2, w3, b3, gamma, beta):
    import ml_dtypes
    nbf = ml_dtypes.bfloat16

    seq = np.asarray(seq, dtype=np.float32)
    bias_mat = np.asarray(bias_mat, dtype=np.float32)
    W1 = np.asarray(W1, dtype=np.float32)
    w2 = np.asarray(w2, dtype=np.float32)
    w3 = np.asarray(w3, dtype=np.float32)

    seqT = np.ascontiguousarray(
        seq.transpose(0, 2, 1).reshape(B, 2, P, N).astype(nbf)
    )
    w1t = W1.T
    f2col = (w1t @ w3).reshape(F, 1)
    w1ext = np.concatenate([w1t, f2col, -f2col], axis=1)
    w1ext = np.ascontiguousarray(w1ext.reshape(2, P, EXT).astype(nbf))
    w1w2 = np.ascontiguousarray((w1t @ w2).reshape(2, P, 1).astype(nbf))

    gam = np.asarray(gamma, dtype=np.float32).reshape(O, 1)
    bet = np.asarray(beta, dtype=np.float32).reshape(O, 1)
    sc = np.array([[float(b2), float(b3)]], dtype=np.float32)

    in_maps = []
    for c in range(R):
        rows = slice(c * NL, (c + 1) * NL)
        bt = bias_mat[:, rows, :].transpose(0, 2, 1)
        bt = np.ascontiguousarray(
            bt.reshape(B, JC, P, NL).transpose(0, 2, 1, 3).astype(nbf)
        )
        sto = np.ascontiguousarray(seqT[:, :, :, rows])
        in_maps.append({
            "seqt": seqT,
            "seqt_own": sto,
            "bias_t": bt,
            "w1ext": w1ext,
            "w1w2": w1w2,
            "gamma_c": gam,
            "beta_c": bet,
            "scalars": sc,
        })
    return in_maps
